# revision 6
# baseline (speedup 1.0000x reference)
"""MultiHeadAttention forward for Trainium2, 8 NeuronCores.

Problem: B=4, S=2048, D=1024, H=16 heads (head_dim 64), fp32.
  qkv = x @ w_qkv + b_qkv ; q *= hd^-0.5
  attn = softmax(q k^T) ; out = (attn v) @ w_out + b_out

Sharding: 4-way data parallel over batch x 2-way tensor parallel over
heads. Core i handles batch i//2, heads 8*(i%2) .. 8*(i%2)+7. Each core
computes a partial output ([2048, 1024]); the two TP halves of each
batch are summed on the host (each core adds b_out/2 so the sum carries
the full bias).

Dataflow per core (all matmuls in float32r - full PE rate, ~1e-4 rel):
  1. x [S, D] -> PE-transpose -> xT [D, S] (channel-major), streamed in
     512-token chunks.
  2. QKV: qT, kT channel-major [512, S] via lhsT=w block, rhs=xT;
     v token-major [S, 512] via lhsT=xT block, rhs=w_v. The 1/8 query
     scaling is folded into w_q/b_q on the host. v is stored per head
     with an appended ones column ([v_h | 1]) so the attention matmul
     also produces the softmax denominator.
  3. Attention per head pair (A at partitions 0:64, B at 64:128):
     scoresT [tk, tq] = kT_blk.T @ qT ; p = exp(scoresT) (no max
     subtraction needed: |scores| <~ 6); attn_psum [65, tq] accumulates
     v_aug.T @ p over the 16 key tiles - row 64 is the denominator.
     Normalize with DVE reciprocal + DRAM-bounce partition broadcast.
  4. out += attn_outT_blk.T @ w_out_blk accumulated over the 4
     128-channel blocks, + b_out/2, written token-major.
"""

import os

import numpy as np

B, S, D, H, HD = 4, 2048, 1024, 16, 64
NCORES = 8
TPW = 2            # tensor-parallel width over heads
HPC = H // TPW     # heads per core
CH = HPC * HD      # q/k/v channels per core (512)
NPAIR = HPC // 2   # head pairs per core
TT = S // 128      # token tiles
KT = D // 128      # contraction tiles for qkv proj
TC = S // 512      # 512-token chunks for qkv phase
SQC = S // 1024    # 1024-token chunks for attention queries

LAST_RESULTS = None
_CACHED = {}


def _build_nc():
    import concourse.bacc as bacc
    import concourse.mybir as mybir
    import concourse.tile as tile

    F32R = mybir.dt.float32r
    F32 = mybir.dt.float32
    EXP = mybir.ActivationFunctionType.Exp

    nc = bacc.Bacc("TRN2", target_bir_lowering=False)

    x = nc.dram_tensor("x", [S, D], F32R, kind="ExternalInput")
    wqkv = nc.dram_tensor("wqkv", [D, 3 * CH], F32R, kind="ExternalInput")
    bqk = nc.dram_tensor("bqk", [2 * CH], F32, kind="ExternalInput")
    bv = nc.dram_tensor("bv", [CH], F32, kind="ExternalInput")
    wout = nc.dram_tensor("wout", [CH, D], F32R, kind="ExternalInput")
    bout = nc.dram_tensor("bout", [D], F32, kind="ExternalInput")
    ident = nc.dram_tensor("ident", [128, 128], F32R, kind="ExternalInput")
    ones = nc.dram_tensor("ones", [1], F32R, kind="ExternalInput")
    o = nc.dram_tensor("o", [S, D], F32, kind="ExternalOutput")

    x4 = x.ap().rearrange("(tt p) d -> tt p d", p=128)      # [TT, 128, D]
    w3 = wqkv.ap().rearrange("(kt p) c -> kt p c", p=128)   # [KT, 128, 3CH]
    wo3 = wout.ap().rearrange("(dt p) c -> dt p c", p=128)  # [4, 128, D]
    o4 = o.ap().rearrange("(tt p) d -> tt p d", p=128)

    CT = CH // 128  # 4 channel tiles per q/k

    with tile.TileContext(nc) as tc:
        with (
            tc.tile_pool(name="persist", bufs=1) as pp,
            tc.tile_pool(name="dram", bufs=4, space="DRAM") as dr,
        ):
            qT = pp.tile([128, CT, S], F32R)        # q^T channel-major
            kT = pp.tile([128, CT, S], F32R)
            # v per (tt, pair): [vA | 1 | vB | 1] -> 130 cols
            vaug = pp.tile([128, TT, NPAIR, 130], F32R)
            bqk_sb = pp.tile([128, 2 * CT], F32)
            bv_sb = pp.tile([128, CH], F32)
            bout_sb = pp.tile([128, D], F32)
            id_sb = pp.tile([128, 128], F32R)

            nc.sync.dma_start(out=id_sb, in_=ident.ap())
            # bqk: [2CH] -> [128 part, 2CT]: bqk[ct*128+p] -> [p, ct]
            nc.sync.dma_start(
                out=bqk_sb, in_=bqk.ap().rearrange("(ct p) -> p ct", p=128))
            nc.gpsimd.dma_start(out=bv_sb, in_=bv.ap().unsqueeze(0).to_broadcast([128, CH]))
            nc.gpsimd.dma_start(out=bout_sb, in_=bout.ap().unsqueeze(0).to_broadcast([128, D]))
            # ones columns of vaug
            vflat = vaug.rearrange("p tt j c -> p (tt j) c")
            nc.gpsimd.dma_start(
                out=vflat[:, :, 64:65],
                in_=ones.ap().unsqueeze(0).to_broadcast([128, TT * NPAIR, 1]))
            nc.gpsimd.dma_start(
                out=vflat[:, :, 129:130],
                in_=ones.ap().unsqueeze(0).to_broadcast([128, TT * NPAIR, 1]))

            # ---------------- Phase A+B: transpose x, QKV projection ----------
            with (
                tc.tile_pool(name="wq", bufs=1) as wq,
                tc.tile_pool(name="xin", bufs=3) as xin,
                tc.tile_pool(name="xtp", bufs=1) as xtp,
                tc.tile_pool(name="psA", bufs=2, space="PSUM") as psA,
                tc.tile_pool(name="psQ", bufs=3, space="PSUM") as psQ,
            ):
                w_sb = wq.tile([128, KT, 3 * CH], F32R)
                for kt in range(KT):
                    nc.sync.dma_start(out=w_sb[:, kt, :], in_=w3[kt])

                for tcn in range(TC):
                    xT = xtp.tile([128, KT, 512], F32R, tag="xT")
                    for ti in range(4):
                        x_in = xin.tile([128, D], F32R, tag="xin")
                        nc.sync.dma_start(out=x_in, in_=x4[tcn * 4 + ti])
                        for ds in range(KT):
                            pst = psA.tile([128, 128], F32R, tag="tp")
                            nc.tensor.transpose(
                                pst, x_in[:, ds * 128:(ds + 1) * 128], id_sb)
                            nc.vector.tensor_copy(
                                out=xT[:, ds, ti * 128:(ti + 1) * 128], in_=pst)
                    # qT / kT (channel-major)
                    for ct in range(2 * CT):
                        ps = psQ.tile([128, 512], F32, tag="qk")
                        for kt in range(KT):
                            nc.tensor.matmul(
                                ps, w_sb[:, kt, ct * 128:(ct + 1) * 128],
                                xT[:, kt, :],
                                start=(kt == 0), stop=(kt == KT - 1))
                        dst = qT if ct < CT else kT
                        nc.vector.tensor_scalar_add(
                            out=dst[:, ct % CT, tcn * 512:(tcn + 1) * 512],
                            in0=ps, scalar1=bqk_sb[:, ct:ct + 1])
                    # v (token-major, pair-packed with ones cols)
                    for ti in range(4):
                        tt = tcn * 4 + ti
                        psv = psQ.tile([128, CH], F32, tag="v")
                        for kt in range(KT):
                            nc.tensor.matmul(
                                psv, xT[:, kt, ti * 128:(ti + 1) * 128],
                                w_sb[:, kt, 2 * CH:3 * CH],
                                start=(kt == 0), stop=(kt == KT - 1))
                        psv4 = psv.rearrange("p (j two c) -> p j two c", two=2, c=64)
                        bv4 = bv_sb.rearrange("p (j two c) -> p j two c", two=2, c=64)
                        nc.vector.tensor_add(
                            out=vaug[:, tt, :, 0:64], in0=psv4[:, :, 0, :],
                            in1=bv4[:, :, 0, :])
                        nc.vector.tensor_add(
                            out=vaug[:, tt, :, 65:129], in0=psv4[:, :, 1, :],
                            in1=bv4[:, :, 1, :])

            # ---------------- Phase C: attention ------------------------------
            with tc.tile_pool(name="aout", bufs=1) as ao:
                attn_outT = ao.tile([128, CT, S], F32R)
                with (
                    tc.tile_pool(name="pT", bufs=3) as pTp,
                    tc.tile_pool(name="rcp", bufs=2) as rcp,
                    tc.tile_pool(name="rb", bufs=2) as rbp,
                    tc.tile_pool(name="psS", bufs=2, space="PSUM") as psS,
                    tc.tile_pool(name="psAt", bufs=4, space="PSUM") as psAt,
                ):
                    for j in range(NPAIR):
                        for qc in range(SQC):
                            q0 = qc * 1024
                            aA = [psAt.tile([65, 512], F32, tag="at", name=f"aA{j}_{qc}_{h2}") for h2 in range(2)]
                            aB = [psAt.tile([65, 512], F32, tag="at", name=f"aB{j}_{qc}_{h2}") for h2 in range(2)]
                            for kt in range(TT):
                                k0 = kt * 128
                                sA = psS.tile([128, 1024], F32, tag="s")
                                for h2 in range(2):
                                    nc.tensor.matmul(
                                        sA[:, h2 * 512:(h2 + 1) * 512],
                                        kT[0:64, j, k0:k0 + 128],
                                        qT[0:64, j, q0 + h2 * 512:q0 + (h2 + 1) * 512],
                                        start=True, stop=True)
                                pTa = pTp.tile([128, 1024], F32R, tag="p")
                                nc.scalar.activation(out=pTa, in_=sA, func=EXP)
                                sB = psS.tile([128, 1024], F32, tag="s")
                                for h2 in range(2):
                                    nc.tensor.matmul(
                                        sB[:, h2 * 512:(h2 + 1) * 512],
                                        kT[64:128, j, k0:k0 + 128],
                                        qT[64:128, j, q0 + h2 * 512:q0 + (h2 + 1) * 512],
                                        start=True, stop=True)
                                pTb = pTp.tile([128, 1024], F32R, tag="p")
                                nc.scalar.activation(out=pTb, in_=sB, func=EXP)
                                for h2 in range(2):
                                    nc.tensor.matmul(
                                        aA[h2], vaug[:, kt, j, 0:65],
                                        pTa[:, h2 * 512:(h2 + 1) * 512],
                                        start=(kt == 0), stop=(kt == TT - 1))
                                for h2 in range(2):
                                    nc.tensor.matmul(
                                        aB[h2], vaug[:, kt, j, 65:130],
                                        pTb[:, h2 * 512:(h2 + 1) * 512],
                                        start=(kt == 0), stop=(kt == TT - 1))
                            # evacuate + normalize
                            rA = rcp.tile([128, 1024], F32, tag="r")
                            rB = rcp.tile([128, 1024], F32, tag="r")
                            for h2 in range(2):
                                nc.vector.reciprocal(
                                    out=rA[64:65, h2 * 512:(h2 + 1) * 512],
                                    in_=aA[h2][64:65, :])
                                nc.vector.reciprocal(
                                    out=rB[64:65, h2 * 512:(h2 + 1) * 512],
                                    in_=aB[h2][64:65, :])
                            dA = dr.tile([1, 1024], F32, tag="d")
                            dB = dr.tile([1, 1024], F32, tag="d")
                            nc.sync.dma_start(out=dA, in_=rA[64:65, :])
                            nc.sync.dma_start(out=dB, in_=rB[64:65, :])
                            rbA = rbp.tile([64, 1024], F32, tag="rb")
                            rbB = rbp.tile([64, 1024], F32, tag="rb")
                            nc.gpsimd.dma_start(out=rbA, in_=dA.to_broadcast([64, 1024]))
                            nc.gpsimd.dma_start(out=rbB, in_=dB.to_broadcast([64, 1024]))
                            for h2 in range(2):
                                c0 = q0 + h2 * 512
                                nc.vector.tensor_mul(
                                    out=attn_outT[0:64, j, c0:c0 + 512],
                                    in0=aA[h2][0:64, :],
                                    in1=rbA[:, h2 * 512:(h2 + 1) * 512])
                                nc.vector.tensor_mul(
                                    out=attn_outT[64:128, j, c0:c0 + 512],
                                    in0=aB[h2][0:64, :],
                                    in1=rbB[:, h2 * 512:(h2 + 1) * 512])

                # ---------------- Phase D: output projection ------------------
                with (
                    tc.tile_pool(name="wo", bufs=1) as wo,
                    tc.tile_pool(name="osb", bufs=3) as osb,
                    tc.tile_pool(name="psD", bufs=4, space="PSUM") as psD,
                ):
                    wo_sb = wo.tile([128, CT, D], F32R)
                    for dt in range(CT):
                        nc.sync.dma_start(out=wo_sb[:, dt, :], in_=wo3[dt])
                    for tt in range(TT):
                        pd = [psD.tile([128, 512], F32, tag="pd", name=f"pd{tt}_{h2}") for h2 in range(2)]
                        for dt in range(CT):
                            for h2 in range(2):
                                nc.tensor.matmul(
                                    pd[h2],
                                    attn_outT[:, dt, tt * 128:(tt + 1) * 128],
                                    wo_sb[:, dt, h2 * 512:(h2 + 1) * 512],
                                    start=(dt == 0), stop=(dt == CT - 1))
                        ot = osb.tile([128, D], F32, tag="o")
                        for h2 in range(2):
                            nc.vector.tensor_add(
                                out=ot[:, h2 * 512:(h2 + 1) * 512], in0=pd[h2],
                                in1=bout_sb[:, h2 * 512:(h2 + 1) * 512])
                        nc.sync.dma_start(out=o4[tt], in_=ot)

    nc.finalize()
    return nc


def _get_nc():
    if "nc" not in _CACHED:
        _CACHED["nc"] = _build_nc()
    return _CACHED["nc"]


def _core_inputs(x, w_qkv, b_qkv, w_out, b_out):
    """Build the 8 per-core input dicts (host-side sharding)."""
    x = np.asarray(x, dtype=np.float32)
    w_qkv = np.asarray(w_qkv, dtype=np.float32)
    b_qkv = np.asarray(b_qkv, dtype=np.float32)
    w_out = np.asarray(w_out, dtype=np.float32)
    b_out = np.asarray(b_out, dtype=np.float32)

    scale = np.float32(HD ** -0.5)
    ident = np.eye(128, dtype=np.float32)
    one = np.ones(1, dtype=np.float32)
    bout_half = (b_out * np.float32(1.0 / TPW)).astype(np.float32)

    in_maps = []
    for core in range(NCORES):
        b, r = divmod(core, TPW)
        cs = slice(CH * r, CH * (r + 1))
        wq = w_qkv[:, 0:D][:, cs] * scale
        wk = w_qkv[:, D:2 * D][:, cs]
        wv = w_qkv[:, 2 * D:3 * D][:, cs]
        w_pack = np.ascontiguousarray(
            np.concatenate([wq, wk, wv], axis=1), dtype=np.float32)
        bqk_pack = np.ascontiguousarray(np.concatenate(
            [b_qkv[0:D][cs] * scale, b_qkv[D:2 * D][cs]]), dtype=np.float32)
        bv_r = np.ascontiguousarray(b_qkv[2 * D:3 * D][cs], dtype=np.float32)
        wout_r = np.ascontiguousarray(w_out[cs, :], dtype=np.float32)
        in_maps.append({
            "x": np.ascontiguousarray(x[b]),
            "wqkv": w_pack,
            "bqk": bqk_pack,
            "bv": bv_r,
            "wout": wout_r,
            "bout": bout_half,
            "ident": ident,
            "ones": one,
        })
    return in_maps


def _ensure_ntff_hook():
    """Register the axon NTFF profile hook (missing antenv.axon_hooks stub)."""
    import sys
    import types

    if "antenv.axon_hooks" in sys.modules:
        return
    try:
        from trn_agent_boot.trn_boot import _ntff_profile_via_ctypes

        hook = _ntff_profile_via_ctypes("/opt/axon/libaxon_pjrt.so")
        if hook is None:
            return
        mod = types.ModuleType("antenv.axon_hooks")
        mod.get_axon_ntff_profile_hook = lambda: hook
        mod.set_axon_ntff_profile_hook = lambda h: None
        sys.modules["antenv.axon_hooks"] = mod
    except Exception:
        pass


def kernel(x, w_qkv, b_qkv, w_out, b_out):
    global LAST_RESULTS
    from concourse.bass_utils import run_bass_kernel_spmd

    nc = _get_nc()
    in_maps = _core_inputs(x, w_qkv, b_qkv, w_out, b_out)
    trace = bool(os.environ.get("BASS_TRACE"))
    if trace:
        _ensure_ntff_hook()
    res = run_bass_kernel_spmd(
        nc, in_maps, core_ids=list(range(NCORES)), trace=trace)
    LAST_RESULTS = res
    out = np.empty((B, S, D), dtype=np.float32)
    for b in range(B):
        out[b] = res.results[TPW * b]["o"]
        for r in range(1, TPW):
            out[b] += res.results[TPW * b + r]["o"]
    return out
